# revision 31
# baseline (speedup 1.0000x reference)
"""Trainium2 Bass kernel for nn_AttnInteractionLayer_16982300688923.

Math: the reference's einsum 'rfdh,rfoh->rfoh' contracts alpha over its own
softmax axis, which sums to exactly 1 — so the whole Q/K/softmax pipeline
collapses to out == vals.  The remaining computation is

    y   = x @ (m*W_v + (1-m)*W_r)          m = sigmoid(mix)  (host-folded)
    y   = relu(y)
    out = (y - mean(y)) * rsqrt(var(y)+eps) * gamma + beta    (LN over last dim)

Sharding: data-parallel over R across 8 cores; weights replicated.  X is
pre-transposed on the host while sharding so the contraction dim lands on
SBUF partitions with fast contiguous DMAs (fp32 has no DMA-transpose path).

Per-core device pipeline (rows = R/8 * F = 16384, fp32):
  DMA x^T slabs -> float32r matmuls accumulate Y[128,512] in PSUM ->
  ACT relu PSUM->SBUF (+sum accum) -> DVE square (+sum-of-squares accum) ->
  batched LN scalar math -> apply (y-mu)*rstd split across GPSIMD/ACT/DVE ->
  DMA out.
Uniform gamma/beta fold into the per-row scalars; per-feature gamma/beta get
two extra broadcast passes (general path).
"""

import numpy as np

R, F, D_IN = 2048, 64, 256
OH = 512  # output_dim * num_head
N_CORES = 8
ROWS_PER_CORE = (R // N_CORES) * F  # 16384
P = 128
BLOCKS = ROWS_PER_CORE // P  # 128
SLAB = 8  # 128-row blocks per slab
N_SLABS = BLOCKS // SLAB  # 16
EPS = 1e-5

# apply-pass engine per block-in-slab: balance GPSIMD / ACT / DVE
import os as _os

APPLY_ENGINES = list(_os.environ.get("K_APPLY", "GGGGGGGG"))
STATS_MODE = _os.environ.get("K_STATS", "bn")  # accum | bnacc | bn

_prog_cache = {}


def _build(affine_mode, g_u, b_u):
    """affine_mode: 'none' (uniform gamma/beta folded into scalars g_u/b_u)
    or 'full' (per-feature gamma/beta tensors applied on device)."""
    from contextlib import ExitStack

    import concourse.bass as bass
    import concourse.mybir as mybir
    import concourse.tile as tile
    from concourse import bacc

    f32 = mybir.dt.float32
    f32r = mybir.dt.float32r
    AF = mybir.ActivationFunctionType
    OP = mybir.AluOpType

    nc = bacc.Bacc(trn_type="TRN2", target_bir_lowering=False)
    # Host-permuted input: [p, s, ko, r] so each partition reads one
    # contiguous 8KB run per slab.
    xt = nc.dram_tensor(
        "xt", [P, N_SLABS, 2, SLAB * P], f32, kind="ExternalInput"
    )
    wc = nc.dram_tensor("w", [D_IN, OH], f32, kind="ExternalInput")
    if affine_mode == "full":
        gam = nc.dram_tensor("gamma", [OH], f32, kind="ExternalInput")
        bet = nc.dram_tensor("beta", [OH], f32, kind="ExternalInput")
    # Host-unpermuted output: [s, p, b, n] so each partition writes one
    # contiguous 16KB run per slab (8KB per half-slab DMA).
    out = nc.dram_tensor(
        "out", [N_SLABS, P, SLAB, OH], f32, kind="ExternalOutput"
    )

    xt_v = xt.bitcast(f32r)
    o_v = out

    with ExitStack() as ctx:
        tc = ctx.enter_context(tile.TileContext(nc))
        const = ctx.enter_context(tc.tile_pool(name="const", bufs=1))
        xin = ctx.enter_context(tc.tile_pool(name="xin", bufs=6))
        psy = ctx.enter_context(tc.tile_pool(name="psy", bufs=6, space="PSUM"))
        yrp = ctx.enter_context(tc.tile_pool(name="yrp", bufs=3))
        sqp = ctx.enter_context(tc.tile_pool(name="sqp", bufs=8))
        stp = ctx.enter_context(tc.tile_pool(name="stp", bufs=8))
        outp = ctx.enter_context(tc.tile_pool(name="outp", bufs=14))

        w_sb = const.tile([P, 2, OH], f32r)
        nc.sync.dma_start(w_sb, wc.rearrange("(ko p) n -> p ko n", p=P).bitcast(f32r))
        eps_sb = const.tile([P, 1], f32)
        nc.vector.memset(eps_sb, EPS)
        if affine_mode == "full":
            g_sb = const.tile([P, OH], f32)
            b_sb = const.tile([P, OH], f32)
            nc.sync.dma_start(
                g_sb, bass.AP(tensor=gam.tensor, offset=gam.offset, ap=[[0, P], *gam.ap])
            )
            nc.sync.dma_start(
                b_sb, bass.AP(tensor=bet.tensor, offset=bet.offset, ap=[[0, P], *bet.ap])
            )

        H = SLAB // 2  # blocks per half-slab stats group
        for s in range(N_SLABS):
            xt_sl = xin.tile([P, 2, SLAB * P], f32r)
            # input rides the Scalar HWDGE queue so it never sits behind the
            # output backlog in the Sync queue's per-engine FIFOs
            nc.scalar.dma_start(xt_sl, xt_v[:, s])
            yr_sl = yrp.tile([P, SLAB, OH], f32)

            for h in range(2):
                mv_h = stp.tile([P, H, 2], f32, tag="mv")
                negmu_h = stp.tile([P, H], f32, tag="negmu")
                rs_h = stp.tile([P, H], f32, tag="rs")
                nm_h = stp.tile([P, H], f32, tag="nm")
                for j in range(H):
                    b = h * H + j
                    py = psy.tile([P, OH], f32)
                    nc.tensor.matmul(
                        py, xt_sl[:, 0, b * P : (b + 1) * P], w_sb[:, 0],
                        start=True, stop=False,
                    )
                    nc.tensor.matmul(
                        py, xt_sl[:, 1, b * P : (b + 1) * P], w_sb[:, 1],
                        start=False, stop=True,
                    )
                    nc.scalar.activation(yr_sl[:, b], py, AF.Relu)
                    st = sqp.tile([P, 6], f32, tag="bnst")
                    nc.vector.bn_stats(st, yr_sl[:, b])
                    nc.vector.bn_aggr(mv_h[:, j], st)

                # Half-slab LN scalar math on [P, H] tiles:
                #   rstd = 1/sqrt(var+eps) (*g_u); nm = -mu*rstd (*g_u + b_u)
                nc.scalar.activation(rs_h, mv_h[:, :, 1], AF.Sqrt, bias=eps_sb)
                nc.vector.reciprocal(rs_h, rs_h)
                nc.vector.tensor_scalar_mul(negmu_h, mv_h[:, :, 0], -1.0)
                nc.vector.tensor_tensor(nm_h, negmu_h, rs_h, OP.mult)
                if affine_mode == "none" and (g_u != 1.0 or b_u != 0.0):
                    if g_u != 1.0:
                        nc.vector.tensor_scalar_mul(rs_h, rs_h, float(g_u))
                    nc.vector.tensor_scalar(
                        nm_h, nm_h, float(g_u), float(b_u), OP.mult, OP.add
                    )

                for q in range(2):
                    oq = outp.tile([P, 2, OH], f32, tag="outq")
                    for j2 in range(2):
                        j = q * 2 + j2
                        b = h * H + j
                        rs_ap = rs_h[:, j : j + 1]
                        nm_ap = nm_h[:, j : j + 1]
                        eng = APPLY_ENGINES[b]
                        ob = oq[:, j2]
                        if eng == "A":
                            nc.scalar.activation(
                                ob, yr_sl[:, b], AF.Identity,
                                bias=nm_ap, scale=rs_ap,
                            )
                        elif eng == "V":
                            nc.vector.tensor_scalar(
                                ob, yr_sl[:, b], rs_ap, nm_ap, OP.mult, OP.add
                            )
                        else:
                            nc.gpsimd.tensor_scalar(
                                ob, yr_sl[:, b], rs_ap, nm_ap, OP.mult, OP.add
                            )
                        if affine_mode == "full":
                            nc.vector.tensor_tensor(ob, ob, g_sb, OP.mult)
                            nc.gpsimd.tensor_tensor(ob, ob, b_sb, OP.add)
                    b0 = h * H + q * 2
                    nc.sync.dma_start(o_v[s, :, b0 : b0 + 2], oq)
    nc.finalize()
    return nc


def _get_prog(affine_mode, g_u, b_u):
    key = (affine_mode, g_u, b_u)
    if key not in _prog_cache:
        _prog_cache[key] = _build(affine_mode, g_u, b_u)
    return _prog_cache[key]


def _prepare(x, W_q, W_k, W_v, W_r, mix, gamma, beta):
    x = np.asarray(x, dtype=np.float32)
    W_v = np.asarray(W_v, dtype=np.float32)
    W_r = np.asarray(W_r, dtype=np.float32)
    gamma = np.asarray(gamma, dtype=np.float32)
    beta = np.asarray(beta, dtype=np.float32)
    m = 1.0 / (1.0 + np.exp(-float(np.asarray(mix).reshape(-1)[0])))
    wc = np.ascontiguousarray((m * W_v + (1.0 - m) * W_r).astype(np.float32))

    if np.all(gamma == gamma.flat[0]) and np.all(beta == beta.flat[0]):
        affine_mode, g_u, b_u = "none", float(gamma.flat[0]), float(beta.flat[0])
    else:
        affine_mode, g_u, b_u = "full", 1.0, 0.0

    x_flat = x.reshape(R * F, D_IN)
    in_maps = []
    for c in range(N_CORES):
        shard = x_flat[c * ROWS_PER_CORE : (c + 1) * ROWS_PER_CORE]
        # [p, s, ko, r] layout: contiguous 8KB per (partition, slab)
        xt_h = np.ascontiguousarray(
            shard.reshape(N_SLABS, SLAB * P, 2, P).transpose(3, 0, 2, 1)
        )
        im = {"xt": xt_h, "w": wc}
        if affine_mode == "full":
            im["gamma"] = gamma
            im["beta"] = beta
        in_maps.append(im)
    return in_maps, affine_mode, g_u, b_u


def _unpermute_out(arr):
    # [s, p, b, n] -> rows ordered (s, b, p)
    return arr.transpose(0, 2, 1, 3).reshape(ROWS_PER_CORE, OH)


def run(trace=False, **inputs):
    """Internal entry: returns (output, BassKernelResults)."""
    from concourse.bass_utils import run_bass_kernel_spmd

    in_maps, affine_mode, g_u, b_u = _prepare(**inputs)
    nc = _get_prog(affine_mode, g_u, b_u)
    res = run_bass_kernel_spmd(nc, in_maps, core_ids=list(range(N_CORES)), trace=trace)
    parts = [
        _unpermute_out(r["out"]).reshape(R // N_CORES, F, OH) for r in res.results
    ]
    return np.concatenate(parts, axis=0), res


def kernel(**inputs):
    out, _ = run(trace=False, **inputs)
    return out


# revision 34
# speedup vs baseline: 1.1264x; 1.1264x over previous
"""Trainium2 Bass kernel for nn_AttnInteractionLayer_16982300688923.

Math: the reference's einsum 'rfdh,rfoh->rfoh' contracts alpha over its own
softmax axis, which sums to exactly 1 — so the whole Q/K/softmax pipeline
collapses to out == vals.  The remaining computation is

    y   = x @ (m*W_v + (1-m)*W_r)          m = sigmoid(mix)  (host-folded)
    y   = relu(y)
    out = (y - mean(y)) * rsqrt(var(y)+eps) * gamma + beta    (LN over last dim)

Sharding: data-parallel over R across 8 cores; weights replicated.  X is
pre-transposed on the host while sharding so the contraction dim lands on
SBUF partitions with fast contiguous DMAs (fp32 has no DMA-transpose path).

Per-core device pipeline (rows = R/8 * F = 16384, fp32):
  DMA x^T slabs -> float32r matmuls accumulate Y[128,512] in PSUM ->
  ACT relu PSUM->SBUF (+sum accum) -> DVE square (+sum-of-squares accum) ->
  batched LN scalar math -> apply (y-mu)*rstd split across GPSIMD/ACT/DVE ->
  DMA out.
Uniform gamma/beta fold into the per-row scalars; per-feature gamma/beta get
two extra broadcast passes (general path).
"""

import numpy as np

R, F, D_IN = 2048, 64, 256
OH = 512  # output_dim * num_head
N_CORES = 8
ROWS_PER_CORE = (R // N_CORES) * F  # 16384
P = 128
BLOCKS = ROWS_PER_CORE // P  # 128
SLAB = 8  # 128-row blocks per slab
N_SLABS = BLOCKS // SLAB  # 16
EPS = 1e-5

# apply-pass engine per block-in-slab: balance GPSIMD / ACT / DVE
import os as _os

APPLY_ENGINES = list(_os.environ.get("K_APPLY", "GGGGGGGG"))
STATS_MODE = _os.environ.get("K_STATS", "bn")  # accum | bnacc | bn

_prog_cache = {}


def _build(affine_mode, g_u, b_u):
    """affine_mode: 'none' (uniform gamma/beta folded into scalars g_u/b_u)
    or 'full' (per-feature gamma/beta tensors applied on device)."""
    from contextlib import ExitStack

    import concourse.bass as bass
    import concourse.mybir as mybir
    import concourse.tile as tile
    from concourse import bacc

    f32 = mybir.dt.float32
    f32r = mybir.dt.float32r
    AF = mybir.ActivationFunctionType
    OP = mybir.AluOpType

    nc = bacc.Bacc(trn_type="TRN2", target_bir_lowering=False)
    # Host-permuted input: [p, s, ko, r] so each partition reads one
    # contiguous 8KB run per slab.
    xt = nc.dram_tensor(
        "xt", [P, N_SLABS, 2, SLAB * P], f32, kind="ExternalInput"
    )
    wc = nc.dram_tensor("w", [D_IN, OH], f32, kind="ExternalInput")
    if affine_mode == "full":
        gam = nc.dram_tensor("gamma", [OH], f32, kind="ExternalInput")
        bet = nc.dram_tensor("beta", [OH], f32, kind="ExternalInput")
    # Host-unpermuted output: [s, p, b, n] so each partition writes one
    # contiguous 16KB run per slab (8KB per half-slab DMA).
    out = nc.dram_tensor(
        "out", [N_SLABS, P, SLAB, OH], f32, kind="ExternalOutput"
    )

    xt_v = xt.bitcast(f32r)
    o_v = out

    with ExitStack() as ctx:
        tc = ctx.enter_context(tile.TileContext(nc))
        const = ctx.enter_context(tc.tile_pool(name="const", bufs=1))
        xin = ctx.enter_context(tc.tile_pool(name="xin", bufs=4))
        psy = ctx.enter_context(tc.tile_pool(name="psy", bufs=4, space="PSUM"))
        yrp = ctx.enter_context(tc.tile_pool(name="yrp", bufs=3))
        sqp = ctx.enter_context(tc.tile_pool(name="sqp", bufs=8))
        stp = ctx.enter_context(tc.tile_pool(name="stp", bufs=8))
        outp = ctx.enter_context(tc.tile_pool(name="outp", bufs=12))

        w_sb = const.tile([P, 2, OH], f32r)
        nc.sync.dma_start(w_sb, wc.rearrange("(ko p) n -> p ko n", p=P).bitcast(f32r))
        eps_sb = const.tile([P, 1], f32)
        nc.vector.memset(eps_sb, EPS)
        if affine_mode == "full":
            g_sb = const.tile([P, OH], f32)
            b_sb = const.tile([P, OH], f32)
            nc.sync.dma_start(
                g_sb, bass.AP(tensor=gam.tensor, offset=gam.offset, ap=[[0, P], *gam.ap])
            )
            nc.sync.dma_start(
                b_sb, bass.AP(tensor=bet.tensor, offset=bet.offset, ap=[[0, P], *bet.ap])
            )

        H = SLAB // 2  # blocks per half-slab stats group
        for s in range(N_SLABS):
            xt_sl = xin.tile([P, 2, SLAB * P], f32r)
            # input rides the Scalar HWDGE queue so it never sits behind the
            # output backlog in the Sync queue's per-engine FIFOs
            nc.scalar.dma_start(xt_sl, xt_v[:, s])
            yr_sl = yrp.tile([P, SLAB, OH], f32)

            for h in range(2):
                mv_h = stp.tile([P, H, 2], f32, tag="mv")
                negmu_h = stp.tile([P, H], f32, tag="negmu")
                rs_h = stp.tile([P, H], f32, tag="rs")
                nm_h = stp.tile([P, H], f32, tag="nm")
                for jj in range(H // 2):
                    b0 = h * H + jj * 2
                    py2 = psy.tile([P, 2, OH], f32)
                    for k in range(2):
                        b = b0 + k
                        nc.tensor.matmul(
                            py2[:, k], xt_sl[:, 0, b * P : (b + 1) * P], w_sb[:, 0],
                            start=True, stop=False,
                        )
                        nc.tensor.matmul(
                            py2[:, k], xt_sl[:, 1, b * P : (b + 1) * P], w_sb[:, 1],
                            start=False, stop=True,
                        )
                    # one relu covers both blocks (2 PSUM banks -> 1024 wide)
                    nc.scalar.activation(yr_sl[:, b0 : b0 + 2], py2, AF.Relu)
                    for k in range(2):
                        j = jj * 2 + k
                        st = sqp.tile([P, 6], f32, tag="bnst")
                        nc.vector.bn_stats(st, yr_sl[:, b0 + k])
                        nc.vector.bn_aggr(mv_h[:, j], st)

                # Half-slab LN scalar math on [P, H] tiles:
                #   rstd = 1/sqrt(var+eps) (*g_u); nm = -mu*rstd (*g_u + b_u)
                nc.scalar.activation(rs_h, mv_h[:, :, 1], AF.Sqrt, bias=eps_sb)
                nc.vector.reciprocal(rs_h, rs_h)
                nc.vector.tensor_scalar_mul(negmu_h, mv_h[:, :, 0], -1.0)
                nc.vector.tensor_tensor(nm_h, negmu_h, rs_h, OP.mult)
                if affine_mode == "none" and (g_u != 1.0 or b_u != 0.0):
                    if g_u != 1.0:
                        nc.vector.tensor_scalar_mul(rs_h, rs_h, float(g_u))
                    nc.vector.tensor_scalar(
                        nm_h, nm_h, float(g_u), float(b_u), OP.mult, OP.add
                    )

                for q in range(2):
                    oq = outp.tile([P, 2, OH], f32, tag="outq")
                    for j2 in range(2):
                        j = q * 2 + j2
                        b = h * H + j
                        rs_ap = rs_h[:, j : j + 1]
                        nm_ap = nm_h[:, j : j + 1]
                        eng = APPLY_ENGINES[b]
                        ob = oq[:, j2]
                        if eng == "A":
                            nc.scalar.activation(
                                ob, yr_sl[:, b], AF.Identity,
                                bias=nm_ap, scale=rs_ap,
                            )
                        elif eng == "V":
                            nc.vector.tensor_scalar(
                                ob, yr_sl[:, b], rs_ap, nm_ap, OP.mult, OP.add
                            )
                        else:
                            nc.gpsimd.tensor_scalar(
                                ob, yr_sl[:, b], rs_ap, nm_ap, OP.mult, OP.add
                            )
                        if affine_mode == "full":
                            nc.vector.tensor_tensor(ob, ob, g_sb, OP.mult)
                            nc.gpsimd.tensor_tensor(ob, ob, b_sb, OP.add)
                    b0 = h * H + q * 2
                    nc.sync.dma_start(o_v[s, :, b0 : b0 + 2], oq)
    nc.finalize()
    return nc


def _get_prog(affine_mode, g_u, b_u):
    key = (affine_mode, g_u, b_u)
    if key not in _prog_cache:
        _prog_cache[key] = _build(affine_mode, g_u, b_u)
    return _prog_cache[key]


def _prepare(x, W_q, W_k, W_v, W_r, mix, gamma, beta):
    x = np.asarray(x, dtype=np.float32)
    W_v = np.asarray(W_v, dtype=np.float32)
    W_r = np.asarray(W_r, dtype=np.float32)
    gamma = np.asarray(gamma, dtype=np.float32)
    beta = np.asarray(beta, dtype=np.float32)
    m = 1.0 / (1.0 + np.exp(-float(np.asarray(mix).reshape(-1)[0])))
    wc = np.ascontiguousarray((m * W_v + (1.0 - m) * W_r).astype(np.float32))

    if np.all(gamma == gamma.flat[0]) and np.all(beta == beta.flat[0]):
        affine_mode, g_u, b_u = "none", float(gamma.flat[0]), float(beta.flat[0])
    else:
        affine_mode, g_u, b_u = "full", 1.0, 0.0

    x_flat = x.reshape(R * F, D_IN)
    in_maps = []
    for c in range(N_CORES):
        shard = x_flat[c * ROWS_PER_CORE : (c + 1) * ROWS_PER_CORE]
        # [p, s, ko, r] layout: contiguous 8KB per (partition, slab)
        xt_h = np.ascontiguousarray(
            shard.reshape(N_SLABS, SLAB * P, 2, P).transpose(3, 0, 2, 1)
        )
        im = {"xt": xt_h, "w": wc}
        if affine_mode == "full":
            im["gamma"] = gamma
            im["beta"] = beta
        in_maps.append(im)
    return in_maps, affine_mode, g_u, b_u


def _unpermute_out(arr):
    # [s, p, b, n] -> rows ordered (s, b, p)
    return arr.transpose(0, 2, 1, 3).reshape(ROWS_PER_CORE, OH)


def run(trace=False, **inputs):
    """Internal entry: returns (output, BassKernelResults)."""
    from concourse.bass_utils import run_bass_kernel_spmd

    in_maps, affine_mode, g_u, b_u = _prepare(**inputs)
    nc = _get_prog(affine_mode, g_u, b_u)
    res = run_bass_kernel_spmd(nc, in_maps, core_ids=list(range(N_CORES)), trace=trace)
    parts = [
        _unpermute_out(r["out"]).reshape(R // N_CORES, F, OH) for r in res.results
    ]
    return np.concatenate(parts, axis=0), res


def kernel(**inputs):
    out, _ = run(trace=False, **inputs)
    return out
